# revision 1
# baseline (speedup 1.0000x reference)
"""Trainium2 Bass kernel for nn_BiClassifier (bilinear pairwise MLP).

Math (per batch b):
    in1 = input1 @ W1.T + b1            # [N1, HID]
    in2 = input2 @ W2.T                 # [N2, HID]
    h   = relu(in1[:,None,:] + in2[None,:,:])   # [N1, N2, HID]  (never materialized)
    out = h @ Wo.T + bo                 # [N1, N2, OUT]

Strategy: shard the 512 (b, n1) rows across 8 cores (64 rows each, one batch
per core pair). Weights are replicated. On each core the work is streamed
per hid-block hp (8 blocks of 128 on the partition axis):
  Phase A(hp) on PE: in1T[:, hp] [128, 64] and in2T[:, hp] [128, 128] from
      host-pre-transposed weights/inputs; the bias-add lands on DVE and the
      bf16 SBUF cast of in2T on ACT (both producer paths read SBUF — ACT
      reading PSUM contends with PE's PSUM writes, measured -8us).
  Phase B(hp): 8 double-tiles h2 [128, 1024] = relu(in2T + in1T[:, n]) for
      group g's 4 rows of BOTH row-supers (one tensor_scalar / activation per
      row, FD=128 — the per-partition scalar structurally caps FD at N2).
      Each h2 feeds two PE matmuls against the same wote [128, 16] stationary
      (Wo embedded at psum rows 2g:2g+2), accumulating [16, 512] PSUM banks
      (one per super) across all hp. Phase A(hp+1) and its aux are emitted
      after g0/g1/g2 so PE prefetches into the producer-limited window
      without stalling the in-order V/ACT queues.
The hp0-critical DMAs are halved and issued in parallel on the two HWDGE
queues (sync + scalar) in first-need order to shorten the ramp (~0.6us issue
per dma_start, ~65GB/s per in-flight transfer).
Steady state: DVE and ACT both run ~100% busy (~60us); PE ~65%. A further
~13.4us is fixed NEFF startup + event-semaphore teardown (a trivial kernel
measures 13.4us wall).
Host pre/post: transpose/shard inputs, unscramble output, add bo.
"""

import sys

import numpy as np

_REPO = "/opt/trn_rl_repo"
if _REPO not in sys.path:
    sys.path.insert(0, _REPO)

import concourse.bass as bass
import concourse.mybir as mybir
import concourse.tile as tile
from concourse import bacc
from concourse.bass_utils import run_bass_kernel_spmd

B, N1, N2, D, HID, OUT = 4, 128, 128, 768, 1024, 2
NCORES = 8
NR = 64            # (b, n1) rows per core
DB = D // 128      # 6 contraction blocks for the input projections
HP = HID // 128    # 8 hid blocks
NSUP = 2           # row supers per core (32 rows each -> one PSUM bank)
NG = 8             # row groups per super
GR = 4             # rows per group (group -> one [128, 512] h half-tile)

# Engine weights (V=Vector, A=Scalar/ACT) for h2-tile production. HW sweep
# optimum: DVE ~163ns and ACT ~310ns effective per [128, 128] quarter; both
# queues are dispatch-bound and saturate together near 66:34.
ADD_W = (66, 34)
# Data dtype for weights/inputs/h tiles. PSUM accumulation and out stay fp32.
DT = "bfloat16"
# Ramp/tail shaping: hp0's in2t cast on DVE; keep hp7's last two slots on
# DVE so the slower ACT queue doesn't end the kernel.
RAMP_CAST_V = True
TAIL_V = True

_CACHE = {}


def _wrr(weights, n):
    """Weighted round-robin schedule of 'V'/'A' over n slots."""
    names = "VA"
    credits = [0.0, 0.0]
    total = float(sum(weights))
    out = []
    for _ in range(n):
        credits = [c + w for c, w in zip(credits, weights)]
        i = max(range(2), key=lambda k: credits[k])
        credits[i] -= total
        out.append(names[i])
    return out


def _build(dt_name=None, add_w=None):
    f32 = mybir.dt.float32
    dt = getattr(mybir.dt, dt_name or DT)
    sched = _wrr(add_w or ADD_W, HP * NG)
    if TAIL_V:
        # Swap any A in hp7's last two slots with the nearest earlier V.
        blk = sched[(HP - 1) * NG :]
        for pos in (NG - 2, NG - 1):
            if blk[pos] == "A":
                for q in range(pos - 1, -1, -1):
                    if blk[q] == "V":
                        blk[q], blk[pos] = blk[pos], blk[q]
                        break
        sched = sched[: (HP - 1) * NG] + blk
    # Bacc (not plain Bass): its finalize() runs the walrus legalization
    # passes (move_matmul_waits_to_ldweights, event semaphores, ...) without
    # which multi-wait instructions fail neuronxcc codegen.
    nc = bacc.Bacc(None, target_bir_lowering=False)

    w1 = nc.declare_dram_parameter("w1", [128, HP * DB * 128], dt, isOutput=False)
    w2 = nc.declare_dram_parameter("w2", [128, HP * DB * 128], dt, isOutput=False)
    wote = nc.declare_dram_parameter("wote", [128, HP * NG * 16], dt, isOutput=False)
    b1s = nc.declare_dram_parameter("b1s", [128, HP], f32, isOutput=False)
    x1 = nc.declare_dram_parameter("x1", [128, DB * NR], dt, isOutput=False)
    x2 = nc.declare_dram_parameter("x2", [128, DB * N2], dt, isOutput=False)
    out = nc.declare_dram_parameter("out", [16, NSUP * 512], f32, isOutput=True)

    with tile.TileContext(nc) as tc:
        with (
            tc.tile_pool(name="const", bufs=1) as cpool,
            tc.tile_pool(name="wpool", bufs=1) as wpool,
            tc.tile_pool(name="hpool", bufs=8) as hpool,
            tc.tile_pool(name="pa1", bufs=2, space=bass.MemorySpace.PSUM) as pa1,
            tc.tile_pool(name="pa2", bufs=2, space=bass.MemorySpace.PSUM) as pa2,
            tc.tile_pool(name="po", bufs=1, space=bass.MemorySpace.PSUM) as po,
        ):
            x1sb = cpool.tile([128, DB * NR], dt)
            # x2 in two half tiles (db 0-2 / 3-5): phase A's first ps2
            # matmuls then wait only on the first half's DMA.
            x2sba = cpool.tile([128, DB // 2 * N2], dt)
            x2sbb = cpool.tile([128, DB // 2 * N2], dt)

            def x2_slice(db):
                t = x2sba if db < DB // 2 else x2sbb
                dbr = db % (DB // 2)
                return t[:, dbr * N2 : (dbr + 1) * N2]
            b1sb = cpool.tile([128, HP], f32)
            wotesb = cpool.tile([128, HP * NG * 16], dt)
            # in1t (fp32) feeds per-partition scalar reads (ACTIVATE bias /
            # tensor_scalar), which are fp32-only.
            in1t = cpool.tile([128, HP * NR], f32)
            in2t = cpool.tile([128, HP * N2], dt)
            outsb = cpool.tile([16, NSUP * 512], f32)

            # Per-hp weight tiles so phase A hp can start as soon as its
            # slice lands. hp0's tiles are split in half so the first
            # matmuls wait only on the first half's DMA.
            HB = DB // 2 * 128
            w1sb = [None]
            w2sb = [None]
            w1sb0 = [
                wpool.tile([128, HB], dt, tag="w1_0a", name="w1_0a"),
                wpool.tile([128, HB], dt, tag="w1_0b", name="w1_0b"),
            ]
            w2sb0 = [
                wpool.tile([128, HB], dt, tag="w2_0a", name="w2_0a"),
                wpool.tile([128, HB], dt, tag="w2_0b", name="w2_0b"),
            ]
            for hp in range(1, HP):
                t1 = wpool.tile([128, DB * 128], dt, tag=f"w1_{hp}")
                t2 = wpool.tile([128, DB * 128], dt, tag=f"w2_{hp}")
                w1sb.append(t1)
                w2sb.append(t2)

            def w_slice(wl, w0, hp, db):
                if hp == 0:
                    t = w0[0] if db < DB // 2 else w0[1]
                    dbr = db % (DB // 2)
                    return t[:, dbr * 128 : (dbr + 1) * 128]
                return wl[hp][:, db * 128 : (db + 1) * 128]

            def _load_w(hp, eng=None):
                (eng or nc.sync).dma_start(
                    out=w1sb[hp][:], in_=w1[:, hp * DB * 128 : (hp + 1) * DB * 128]
                )
                (eng or nc.sync).dma_start(
                    out=w2sb[hp][:], in_=w2[:, hp * DB * 128 : (hp + 1) * DB * 128]
                )

            # DMA fill: the ramp is bounded by issue cost (~0.6us per
            # dma_start, serial per queue) plus ~1.5us transfer per 98KB
            # chunk, so the hp0-critical tiles are split in half and issued
            # in parallel across the two HWDGE queues (sync + scalar) in
            # first-need order (ps1: x1+w1[0]; ps2: x2+w2[0]; then b1/wote).
            # Third queue: gpsimd software-DGE DMA measured at ~70GB/s, same
            # as the HWDGE queues — spread the 684KB of hp0-critical bytes
            # over three queues instead of two.
            HX2 = DB // 2 * N2
            nc.sync.dma_start(out=x2sba[:], in_=x2[:, :HX2])
            nc.scalar.dma_start(out=x2sbb[:], in_=x2[:, HX2 : DB * N2])
            nc.gpsimd.dma_start(out=x1sb[:], in_=x1[:])
            nc.sync.dma_start(out=w1sb0[0][:], in_=w1[:, :HB])
            nc.scalar.dma_start(out=w2sb0[0][:], in_=w2[:, :HB])
            nc.gpsimd.dma_start(out=w2sb0[1][:], in_=w2[:, HB : DB * 128])
            nc.sync.dma_start(out=w1sb0[1][:], in_=w1[:, HB : DB * 128])
            nc.scalar.dma_start(out=b1sb[:], in_=b1s[:])

            # Dummy activation: pulls the ~1.3us ACT table load into the DMA
            # fill window instead of the first real relu.
            warm = cpool.tile([128, 1], f32, name="warm")
            nc.vector.memset(warm[:], 0.0)
            nc.scalar.activation(
                warm[:], warm[:], mybir.ActivationFunctionType.Relu, bias=0.0,
                scale=1.0,
            )

            # wote on the gpsimd queue: it is free after its two ramp chunks,
            # while the sync queue still has ~292KB queued ahead — this gets
            # wote to the first pso matmul (~11.5us) without a PE stall.
            nc.gpsimd.dma_start(out=wotesb[:], in_=wote[:])
            _load_w(1)
            for hp in range(2, HP):
                _load_w(hp)

            # Output accumulators: one [16, 512] bank per row-super, live
            # for the whole phase-B accumulation (all hp).
            pso = [po.tile([16, 512], f32, name=f"pso{s}") for s in range(NSUP)]

            def phase_a(hp):
                ps1 = pa1.tile([128, NR], f32, tag="ps1")
                for db in range(DB):
                    nc.tensor.matmul(
                        ps1[:],
                        w_slice(w1sb, w1sb0, hp, db),
                        x1sb[:, db * NR : (db + 1) * NR],
                        start=(db == 0),
                        stop=(db == DB - 1),
                    )
                ps2 = pa2.tile([128, N2], f32, tag="ps2")
                for db in range(DB):
                    nc.tensor.matmul(
                        ps2[:],
                        w_slice(w2sb, w2sb0, hp, db),
                        x2_slice(db),
                        start=(db == 0),
                        stop=(db == DB - 1),
                    )
                return ps1, ps2

            def a_aux_v(hp, ps1):
                # in1t slice (fp32, += b1) on DVE.
                nc.vector.tensor_scalar_add(
                    in1t[:, hp * NR : (hp + 1) * NR], ps1[:], b1sb[:, hp : hp + 1]
                )

            def a_aux_a(hp, ps2):
                # bf16 SBUF copy of in2T on ACT.
                nc.scalar.copy(in2t[:, hp * N2 : (hp + 1) * N2], ps2[:])

            def a_aux(hp, ps1, ps2):
                a_aux_v(hp, ps1)
                a_aux_a(hp, ps2)

            cur = phase_a(0)
            a_aux_v(0, cur[0])
            if RAMP_CAST_V:
                # hp0's cast on DVE: ACT's queue is still busy with DMA
                # issues + the warm-up table load during the ramp.
                nc.vector.tensor_copy(in2t[:, 0:N2], cur[1][:])
            else:
                a_aux_a(0, cur[1])

            # Phase B, natural WRR interleave of V/A tiles. (Measured dead
            # ends: A-first ordering -6us — the in-order pso chain convoys
            # behind slow A tiles; double-width V tiles -3us — same convoy
            # through the 16-quarter wait; 8-wide stationaries with split
            # [8, 512] banks -15us — small-output-partition matmuls are
            # pathological on PE.)
            for hp in range(HP):
                src = in2t[:, hp * N2 : (hp + 1) * N2]
                for g in range(NG):
                    which = sched[hp * NG + g]
                    h2 = hpool.tile([128, NSUP * GR * N2], dt, tag="h1")
                    for sup in range(NSUP):
                        for j in range(GR):
                            row = sup * (NG * GR) + g * GR + j
                            col = in1t[:, hp * NR + row : hp * NR + row + 1]
                            dst = h2[:, (sup * GR + j) * N2 : (sup * GR + j + 1) * N2]
                            if which == "A":
                                # src from SBUF (not PSUM): ACT's PSUM reads
                                # contend with PE's PSUM writes.
                                nc.scalar.activation(
                                    dst,
                                    src,
                                    mybir.ActivationFunctionType.Relu,
                                    bias=col,
                                    scale=1.0,
                                )
                            else:
                                nc.vector.tensor_scalar(
                                    dst,
                                    src,
                                    col,
                                    0.0,
                                    mybir.AluOpType.add,
                                    mybir.AluOpType.max,
                                )
                    wslice = wotesb[
                        :, (hp * NG + g) * 16 : (hp * NG + g + 1) * 16
                    ]
                    for sup in range(NSUP):
                        nc.tensor.matmul(
                            pso[sup][:],
                            wslice,
                            h2[:, sup * GR * N2 : (sup + 1) * GR * N2],
                            start=(hp == 0 and g == 0),
                            stop=(hp == HP - 1 and g == NG - 1),
                        )
                    if hp + 1 < HP:
                        if g == 0:
                            # Prefetch next hp's projections into the PE
                            # bubble; producers still have ~7 groups queued.
                            cur = phase_a(hp + 1)
                        elif g == 1:
                            # Aux lands on V/ACT later so their in-order
                            # queues don't stall on ps1/ps2 of the prefetch
                            # (which waits on slot 0's matmuls).
                            a_aux_v(hp + 1, cur[0])
                        elif g == 2:
                            a_aux_a(hp + 1, cur[1])

            # Evacuate on both engines in parallel to shorten the tail.
            nc.vector.tensor_copy(outsb[:, 0:512], pso[0][:])
            nc.scalar.copy(outsb[:, 512:1024], pso[1][:])
            nc.sync.dma_start(out=out[:], in_=outsb[:])

    nc.finalize()
    return nc


def _np_dt(dt_name):
    if dt_name == "bfloat16":
        import ml_dtypes

        return ml_dtypes.bfloat16
    return np.float32


def _host_prep(input1, input2, W1, b1, W2, Wo, dt_name=None):
    f32 = np.float32
    dt = _np_dt(dt_name or DT)
    c = np.ascontiguousarray

    # w[p, hp, db, j] = W[hp*128+j, db*128+p]
    w1sb = c(W1.reshape(HP, 128, DB, 128).transpose(3, 0, 2, 1).reshape(128, -1), dt)
    w2sb = c(W2.reshape(HP, 128, DB, 128).transpose(3, 0, 2, 1).reshape(128, -1), dt)

    # wote[p, hp, s, 2s+o] = Wo[o, hp*128+p]
    wo_hpo = Wo.T.reshape(HP, 128, OUT)  # [hp, p, o]
    wote = np.zeros((128, HP, NG, 16), f32)
    for s in range(NG):
        wote[:, :, s, 2 * s : 2 * s + 2] = wo_hpo.transpose(1, 0, 2)
    wote = c(wote.reshape(128, -1), dt)

    b1sb = c(b1.reshape(HP, 128).T, f32)

    in_maps = []
    for core in range(NCORES):
        b, half = core // 2, core % 2
        n0 = half * NR
        x1sb = c(
            input1[b, n0 : n0 + NR].reshape(NR, DB, 128).transpose(2, 1, 0).reshape(128, -1),
            dt,
        )
        x2sb = c(
            input2[b].reshape(N2, DB, 128).transpose(2, 1, 0).reshape(128, -1), dt
        )
        in_maps.append(
            {"w1": w1sb, "w2": w2sb, "wote": wote, "b1s": b1sb, "x1": x1sb, "x2": x2sb}
        )
    return in_maps


def _host_post(results, bo):
    out_full = np.empty((B, N1, N2, OUT), np.float32)
    for core in range(NCORES):
        b, half = core // 2, core % 2
        co = np.asarray(results[core]["out"], np.float32)
        co = co.reshape(NG, OUT, NSUP, GR, N2)  # [s, o, sup, j, m]
        arr = co.transpose(2, 0, 3, 4, 1).reshape(NR, N2, OUT)  # [sup,s,j] -> rows
        out_full[b, half * NR : (half + 1) * NR] = arr
    out_full += np.asarray(bo, np.float32)
    return out_full


def run(inputs, trace=False, dt_name=None, add_w=None, **spmd_kwargs):
    """Run on hardware; returns (output, BassKernelResults)."""
    key = (dt_name or DT, add_w or ADD_W)
    if key not in _CACHE:
        _CACHE[key] = _build(dt_name=dt_name, add_w=add_w)
    nc = _CACHE[key]
    in_maps = _host_prep(
        np.asarray(inputs["input1"], np.float32),
        np.asarray(inputs["input2"], np.float32),
        np.asarray(inputs["W1"], np.float32),
        np.asarray(inputs["b1"], np.float32),
        np.asarray(inputs["W2"], np.float32),
        np.asarray(inputs["Wo"], np.float32),
        dt_name=dt_name,
    )
    res = run_bass_kernel_spmd(
        nc, in_maps, list(range(NCORES)), trace=trace, **spmd_kwargs
    )
    out = _host_post(res.results, np.asarray(inputs["bo"], np.float32))
    return out, res


def kernel(**inputs) -> np.ndarray:
    out, _ = run(inputs, trace=False)
    return out


if __name__ == "__main__":
    rng = np.random.default_rng(0)
    ins = {
        "input1": rng.standard_normal((B, N1, D), dtype=np.float32),
        "input2": rng.standard_normal((B, N2, D), dtype=np.float32),
        "W1": rng.standard_normal((HID, D), dtype=np.float32) * 0.036,
        "b1": rng.standard_normal((HID,), dtype=np.float32) * 0.036,
        "W2": rng.standard_normal((HID, D), dtype=np.float32) * 0.036,
        "Wo": rng.standard_normal((OUT, HID), dtype=np.float32) * 0.031,
        "bo": rng.standard_normal((OUT,), dtype=np.float32) * 0.031,
    }
    out = kernel(**ins)
    print("kernel out", out.shape, out.dtype)



# revision 2
# speedup vs baseline: 1.0025x; 1.0025x over previous
"""Trainium2 Bass kernel for nn_BiClassifier (bilinear pairwise MLP).

Math (per batch b):  out[n,m,:] = Wo @ relu(in1[n] + in2[m]) + bo
with in1 = input1 @ W1.T + b1 (HID=1024), in2 = input2 @ W2.T.

Identity used:       relu(a + c) = max(a, -c) + c
  => out = Wo @ max(a, -c)  +  v(m)  + bo,   v = input2 @ (Wo @ W2).T
The v term is rank-structured and computed exactly on the host (a [2,768]
matmul); the device only computes the irreducible pairwise part.

Sharding: HID is split across 8 cores (128 each; phase A per core is just
12 matmuls against 768-wide weight slices). Every core sees the full
inputs, produces all 4*128*128 pairs of max(a,-c) for its h-slice,
projects them against its Wo slice, and returns a [128, 1024] bf16
partial that the host sums over cores and un-diagonalizes.

Pairwise gen (the machine bottleneck: 8.4M elements/core through 128-lane
engines) is split across engines by n-row ranges:
 - n < 112: DVE tensor_tensor(max) over DIAGONALS of the (n,m) grid. With
   in2 negated and duplicated ([-c|-c], 256 cols), diagonal d is
   max(a[:, n], negdbl[:, d+n]): both operands have stride-1 innermost
   access (a: outer stride 0 over d; c: outer stride 1, overlapping
   windows), which qualifies for the DVE 2x_1P bf16 mode at FD=14336 (one
   instruction per batch; ~61ns per 128-pair row vs ~163ns for the
   baseline's per-row tensor_scalar whose per-partition scalar caps FD
   at 128).
 - n >= 112: ACT relu rows (activation Relu, in_=negdbl with scale=-1,
   bias=fp32 a-column), filling the otherwise idle ACT queue. These rows
   include the +c term, so the host applies v only to n < 112.
 (GPSIMD row gen measured pathological: ~2.8us/row tensor_scalar.)

Projection: per 512-pair chunk, matmul with a [128, 32] Wo-embedding
stationary (slot s at cols 2s:2s+2 -> psum rows 32*j4+2s:+2). Four
32-partition column groups of one [128, 512] PSUM bank are filled
concurrently via tile_position=(0, 32*j4) (4-way PE column tiling,
j4 = chunk%4 so consecutive matmuls hit different col groups), 16 slots
per group; half-bank ACT drains stream out as bf16 via both HWDGE queues,
overlapped with compute. Phase A for all batches and the ACT row-gen are
emitted up front so they execute inside the DMA-ramp window and the DVE
runs a gapless TT stream; the last batch's TT is split (96+32 diagonals)
to shorten the PE tail.

Host pre/post: transpose/shard inputs and weights (bf16), build the Wo
embeddings, un-diagonalize the output, add v and bo in fp32.
"""

import sys

import numpy as np

_REPO = "/opt/trn_rl_repo"
if _REPO not in sys.path:
    sys.path.insert(0, _REPO)

import concourse.bass as bass
import concourse.mybir as mybir
import concourse.tile as tile
from concourse import bacc
from concourse.ap import AP
from concourse.bass_utils import run_bass_kernel_spmd

B, N, D, HID, OUT = 4, 128, 768, 1024, 2
NCORES = 8
DB = D // 128           # 6 contraction blocks for phase A
G = 64                  # diagonals per tensor_tensor instruction
NTT = N // G            # TT instructions per batch
R_ACT = 16              # rows per batch generated on ACT (relu-row mode)
R_GPS = 0               # rows per batch on GPSIMD (measured pathological:
                        # gpsimd tensor_scalar with AP scalar is ~2.8us/row)
NE = N - R_ACT - R_GPS  # n-extent covered by the diagonal TTs
MM_PER_TT = G * NE // 512  # projection matmuls per TT
SLOTS = 16              # Wo-embedding slots per PSUM bank
NCHUNKS = B * N * N // 512      # 128 chunks of 512 pairs
DT = "bfloat16"

_CACHE = {}


def _build():
    f32 = mybir.dt.float32
    dt = getattr(mybir.dt, DT)
    nc = bacc.Bacc(None, target_bir_lowering=False)

    w1 = nc.declare_dram_parameter("w1", [128, DB * 128], dt, isOutput=False)
    w2 = nc.declare_dram_parameter("w2", [128, DB * 128], dt, isOutput=False)
    x1 = nc.declare_dram_parameter("x1", [128, B * DB * 128], dt, isOutput=False)
    x2 = nc.declare_dram_parameter("x2", [128, B * DB * 128], dt, isOutput=False)
    wote = nc.declare_dram_parameter("wote", [128, SLOTS * 2 * SLOTS], dt, isOutput=False)
    b1s = nc.declare_dram_parameter("b1s", [128, 1], f32, isOutput=False)
    out = nc.declare_dram_parameter("out", [128, 2 * 512], dt, isOutput=True)

    with tile.TileContext(nc) as tc:
        with (
            tc.tile_pool(name="const", bufs=1) as cpool,
            tc.tile_pool(name="hpool", bufs=3) as hpool,
            tc.tile_pool(name="apool", bufs=3) as apool,
            tc.tile_pool(name="gpool", bufs=3) as gpool,
            tc.tile_pool(name="pa1", bufs=2, space=bass.MemorySpace.PSUM) as pa1,
            tc.tile_pool(name="pa2", bufs=2, space=bass.MemorySpace.PSUM) as pa2,
            tc.tile_pool(name="po", bufs=2, space=bass.MemorySpace.PSUM) as po,
        ):
            w1sb = cpool.tile([128, DB * 128], dt)
            w2sb = cpool.tile([128, DB * 128], dt)
            # x split per batch so phase A(b) starts as soon as batch b lands.
            x1sb = cpool.tile([128, B * DB * 128], dt)
            x2sb = cpool.tile([128, B * DB * 128], dt)
            wotesb = cpool.tile([128, SLOTS * 2 * SLOTS], dt)
            b1sb = cpool.tile([128, 1], f32)
            in1t = cpool.tile([128, B * N], dt)        # a + b1, [h, (b, n)]
            RT = R_ACT + R_GPS
            in1tf = cpool.tile([128, B * RT], f32)  # fp32 a-cols for row modes
            negdbl = cpool.tile([128, B * 2 * N], dt)  # [-c | -c] per batch
            outsb = cpool.tile([128, 2 * 512], dt)

            # Ramp-critical DMAs (w1, w2, x1b0, x2b0 — everything phase A(0)
            # needs) ride the two fast HWDGE queues, 392KB each, halves
            # interleaved so the ps1 inputs land before the ps2 inputs.
            # gpsimd's slow SWDGE (~20GB/s) only carries b1s/wote/wot2,
            # which aren't needed until the first projection matmul.
            XB = DB * 128  # 768 cols per batch
            XH = XB // 2
            nc.scalar.dma_start(out=b1sb[:], in_=b1s[:])
            nc.sync.dma_start(out=w1sb[:], in_=w1[:])
            nc.scalar.dma_start(out=w2sb[:], in_=w2[:])
            nc.sync.dma_start(out=x1sb[:, 0:XH], in_=x1[:, 0:XH])
            nc.scalar.dma_start(out=x1sb[:, XH:XB], in_=x1[:, XH:XB])
            nc.sync.dma_start(out=x2sb[:, 0:XH], in_=x2[:, 0:XH])
            nc.scalar.dma_start(out=x2sb[:, XH:XB], in_=x2[:, XH:XB])
            nc.sync.dma_start(out=wotesb[:], in_=wote[:])

            # Warm the ACT table (Copy set) during the DMA fill window.
            warm = cpool.tile([128, 1], f32, name="warm")
            nc.vector.memset(warm[:], 0.0)
            nc.scalar.mul(warm[:], warm[:], -1.0)

            def phase_a(b):
                ps1 = pa1.tile([128, N], f32, tag="ps1")
                for db in range(DB):
                    nc.tensor.matmul(
                        ps1[:],
                        w1sb[:, db * 128 : (db + 1) * 128],
                        x1sb[:, (b * DB + db) * 128 : (b * DB + db + 1) * 128],
                        start=(db == 0),
                        stop=(db == DB - 1),
                    )
                ps2 = pa2.tile([128, N], f32, tag="ps2")
                for db in range(DB):
                    nc.tensor.matmul(
                        ps2[:],
                        w2sb[:, db * 128 : (db + 1) * 128],
                        x2sb[:, (b * DB + db) * 128 : (b * DB + db + 1) * 128],
                        start=(db == 0),
                        stop=(db == DB - 1),
                    )
                return ps1, ps2

            def aux(b, ps1, ps2):
                # All evacs on ACT so DVE runs a near-pure TT stream:
                # in1t slice (bf16, += b1); negated doubled in2 written as a
                # single FD=256 op (psum source read twice via 0-stride dim).
                nc.scalar.add(in1t[:, b * N : (b + 1) * N], ps1[:], b1sb[:, 0:1])
                nc.scalar.add(
                    in1tf[:, b * RT : (b + 1) * RT],
                    ps1[:, NE:N],
                    b1sb[:, 0:1],
                )
                d_sl = negdbl[:, b * 2 * N : (b + 1) * 2 * N]
                d_ap = AP(d_sl.tensor, d_sl.offset,
                          [list(d_sl.ap[0]), [N, 2], [1, N]])
                s_sl = ps2[:]
                s_ap = AP(s_sl.tensor, s_sl.offset,
                          [list(s_sl.ap[0]), [0, 2], [1, N]])
                nc.scalar.mul(d_ap, s_ap, -1.0)

            def act_gen(b):
                # R_ACT relu rows (n >= NE+R_GPS) on ACT: relu(c + a_n) read
                # from negdbl with scale=-1, bias = fp32 a-column.
                at = apool.tile([128, R_ACT * N], dt, tag="act")
                for k in range(R_ACT):
                    i = b * RT + R_GPS + k
                    nc.scalar.activation(
                        at[:, k * N : (k + 1) * N],
                        negdbl[:, b * 2 * N : b * 2 * N + N],
                        mybir.ActivationFunctionType.Relu,
                        bias=in1tf[:, i : i + 1],
                        scale=-1.0,
                    )
                return at

            def gps_gen(b):
                # R_GPS max rows (NE <= n < NE+R_GPS) on GPSIMD:
                # max(-c, a_n) — same max semantics as the diagonals.
                if not R_GPS:
                    return None
                gt = gpool.tile([128, R_GPS * N], dt, tag="gps")
                for k in range(R_GPS):
                    i = b * RT + k
                    nc.gpsimd.tensor_scalar(
                        gt[:, k * N : (k + 1) * N],
                        negdbl[:, b * 2 * N : b * 2 * N + N],
                        in1tf[:, i : i + 1],
                        None,
                        mybir.AluOpType.max,
                    )
                return gt

            # Phase A for all batches runs entirely inside the DMA-ramp
            # window (PE is otherwise idle there), so the main loop is a
            # pure TT -> projection pipeline. The non-critical x DMAs are
            # emitted after aux(0) so their issue cost doesn't sit ahead of
            # aux(0) in the in-order ACT queue.
            act_tiles = {}
            gps_tiles = {}
            cur = phase_a(0)
            aux(0, *cur)
            act_tiles[0] = act_gen(0)
            gps_tiles[0] = gps_gen(0)
            nc.sync.dma_start(out=x1sb[:, XB : 2 * XB], in_=x1[:, XB : 2 * XB])
            nc.scalar.dma_start(out=x2sb[:, XB : 2 * XB], in_=x2[:, XB : 2 * XB])
            nc.sync.dma_start(out=x2sb[:, 2 * XB :], in_=x2[:, 2 * XB :])
            nc.scalar.dma_start(out=x1sb[:, 2 * XB :], in_=x1[:, 2 * XB :])
            for b in range(1, B):
                cur = phase_a(b)
                aux(b, *cur)
                act_tiles[b] = act_gen(b)
                gps_tiles[b] = gps_gen(b)

            # Projection: one [128, 512] PSUM bank per 64 chunks, split into
            # four 32-partition column groups (tile_position from the psum
            # slice base) so four M=32 matmuls run concurrently in the PE
            # array. chunk c: bank t=c//64, colgroup j4=(c%64)%4 (j4 fastest
            # so consecutive matmuls hit different col groups), slot s=
            # (c%64)//4 picks the Wo embedding and psum rows 32*j4+2*s:+2.
            c_global = 0
            pso = None

            def project(src_ap, cc63_drain=True):
                nonlocal c_global, pso
                t, cc = c_global // 64, c_global % 64
                j4, s = cc % 4, cc // 4
                if cc == 0:
                    pso = po.tile([128, 512], f32, tag="pso")
                nc.tensor.matmul(
                    pso[32 * j4 : 32 * (j4 + 1), :],
                    wotesb[:, s * 2 * SLOTS : (s + 1) * 2 * SLOTS],
                    src_ap,
                    start=(s == 0),
                    stop=(s == SLOTS - 1),
                    tile_position=(0, 32 * j4),
                )
                return t, cc

            def drain(t, half):
                # Half-bank drain (col groups 0-1 after cc==61, 2-3 after
                # cc==63) + its DMA, overlapped with the remaining compute.
                p0, p1 = (0, 64) if half == 0 else (64, 128)
                q = nc.sync if half == 0 else nc.scalar
                nc.scalar.copy(
                    outsb[p0:p1, t * 512 : (t + 1) * 512],
                    pso[p0:p1, :],
                )
                q.dma_start(
                    out=out[p0:p1, t * 512 : (t + 1) * 512],
                    in_=outsb[p0:p1, t * 512 : (t + 1) * 512],
                )

            # Per batch: the ACT-generated chunks are consumed FIRST (their
            # gen runs early on the otherwise-idle ACT queue), so the final
            # chunks of the kernel come straight from the last TT and the
            # stream never ends waiting on ACT.
            for b in range(B):
                at = act_tiles[b]
                for j in range(R_ACT * N // 512):
                    t, cc = project(at[:, j * 512 : (j + 1) * 512])
                    if cc == 61:
                        drain(t, 0)
                    elif cc == 63:
                        drain(t, 1)
                    c_global += 1
                gt = gps_tiles[b]
                for j in range(R_GPS * N // 512):
                    t, cc = project(gt[:, j * 512 : (j + 1) * 512])
                    if cc == 61:
                        drain(t, 0)
                    elif cc == 63:
                        drain(t, 1)
                    c_global += 1
                # One big TT per batch; the very last is split so the PE
                # tail after the final TT is short.
                segs = [(0, 96), (96, 32)] if b == B - 1 else [(0, N)]
                for d0, g in segs:
                    gen = hpool.tile([128, N * NE], dt, tag="gen",
                                     name="gen")[:, : g * NE]
                    a_sl = in1t[:, b * N : b * N + NE]
                    a_ap = AP(
                        a_sl.tensor, a_sl.offset,
                        [list(a_sl.ap[0]), [0, g], [1, NE]],
                    )
                    c_sl = negdbl[:, b * 2 * N + d0 : b * 2 * N + d0 + NE]
                    c_ap = AP(
                        c_sl.tensor, c_sl.offset,
                        [list(c_sl.ap[0]), [1, g], [1, NE]],
                    )
                    g_ap = AP(
                        gen.tensor, gen.offset,
                        [list(gen.ap[0]), [NE, g], [1, NE]],
                    )
                    nc.vector.tensor_tensor(g_ap, a_ap, c_ap, mybir.AluOpType.max)
                    for j in range(g * NE // 512):
                        t, cc = project(gen[:, j * 512 : (j + 1) * 512])
                        if cc == 61:
                            drain(t, 0)
                        elif cc == 63:
                            drain(t, 1)
                        c_global += 1

    nc.finalize()
    return nc


def _np_dt():
    import ml_dtypes

    return ml_dtypes.bfloat16


def _host_prep(input1, input2, W1, b1, W2, Wo):
    f32 = np.float32
    dt = _np_dt()
    c = np.ascontiguousarray

    x1 = c(input1.reshape(B, N, DB, 128).transpose(3, 0, 2, 1).reshape(128, -1), dt)
    x2 = c(input2.reshape(B, N, DB, 128).transpose(3, 0, 2, 1).reshape(128, -1), dt)
    in_maps = []
    for core in range(NCORES):
        h0 = core * 128
        w1k = c(W1[h0:h0+128].reshape(128, DB, 128).transpose(2, 1, 0).reshape(128, -1), dt)
        w2k = c(W2[h0:h0+128].reshape(128, DB, 128).transpose(2, 1, 0).reshape(128, -1), dt)
        wotek = np.zeros((128, SLOTS, 2 * SLOTS), f32)
        for s in range(SLOTS):
            wotek[:, s, 2 * s] = Wo[0, h0:h0+128]
            wotek[:, s, 2 * s + 1] = Wo[1, h0:h0+128]
        in_maps.append({
            "w1": w1k, "w2": w2k, "x1": x1, "x2": x2,
            "wote": c(wotek.reshape(128, -1), dt),
            "b1s": c(b1[h0:h0+128].reshape(128, 1), f32),
        })
    return in_maps


_POST_IDX = None


def _post_indices():
    global _POST_IDX
    if _POST_IDX is None:
        cc = np.arange(NCHUNKS)
        col = np.arange(512)
        nact = R_ACT * N // 512
        ngps = R_GPS * N // 512
        per_b = nact + ngps + NTT * MM_PER_TT
        b_l, n_l, m_l = [], [], []
        for c in range(NCHUNKS):
            b, local = c // per_b, c % per_b
            if local < nact:
                n = NE + R_GPS + (local * 512 + col) // N
                m = col % N
            elif local < nact + ngps:
                n = NE + ((local - nact) * 512 + col) // N
                m = col % N
            else:
                local2 = local - nact - ngps
                tt, j = local2 // MM_PER_TT, local2 % MM_PER_TT
                flat = j * 512 + col
                dl, n = flat // NE, flat % NE
                m = (n + tt * G + dl) % N
            b_l.append(np.full(512, b))
            n_l.append(n)
            m_l.append(m)
        b_full, n_idx, m_idx = np.array(b_l), np.array(n_l), np.array(m_l)
        # psum location: value(c, col, o) = summed[32*j4 + 2*s + o, t*512+col]
        t_i, c64 = cc // 64, cc % 64
        j4_i, s_i = c64 % 4, c64 // 4
        row0 = 32 * j4_i + 2 * s_i
        src_col = t_i[:, None] * 512 + col[None, :]
        _POST_IDX = (b_full, n_idx, m_idx, row0, src_col)
    return _POST_IDX


def _host_post(results, varr, bo):
    summed = np.zeros((128, 2 * 512), np.float32)
    for core in range(NCORES):
        summed += np.asarray(results[core]["out"], np.float32)

    b_full, n_idx, m_idx, row0, src_col = _post_indices()
    vals = np.stack(
        [summed[row0[:, None], src_col], summed[row0[:, None] + 1, src_col]],
        axis=-1,
    )
    out_full = np.empty((B, N, N, OUT), np.float32)
    out_full[b_full, n_idx, m_idx] = vals
    # v correction (v = in2 @ (Wo @ W2).T, exact, host-side) applies to the
    # max-semantics diag rows; the ACT relu rows already include the +c term.
    out_full[:, : NE + R_GPS] += varr[:, None, :, :]
    out_full += np.asarray(bo, np.float32)
    return out_full


def run(inputs, trace=False, **spmd_kwargs):
    spmd_kwargs.pop("dt_name", None)
    spmd_kwargs.pop("add_w", None)
    if "nc" not in _CACHE:
        _CACHE["nc"] = _build()
    nc = _CACHE["nc"]
    in_maps = _host_prep(
        np.asarray(inputs["input1"], np.float32),
        np.asarray(inputs["input2"], np.float32),
        np.asarray(inputs["W1"], np.float32),
        np.asarray(inputs["b1"], np.float32),
        np.asarray(inputs["W2"], np.float32),
        np.asarray(inputs["Wo"], np.float32),
    )
    res = run_bass_kernel_spmd(
        nc, in_maps, list(range(NCORES)), trace=trace, **spmd_kwargs
    )
    # v[b, m, o] = sum_h Wo[o, h] * (input2 @ W2.T)[b, m, h]
    #            = input2 @ (Wo @ W2).T  — tiny, exact, on the host.
    vmat = np.asarray(inputs["Wo"], np.float32) @ np.asarray(inputs["W2"], np.float32)
    varr = np.einsum(
        "bmd,od->bmo", np.asarray(inputs["input2"], np.float32), vmat
    )
    out = _host_post(res.results, varr, np.asarray(inputs["bo"], np.float32))
    return out, res


def kernel(**inputs) -> np.ndarray:
    out, _ = run(inputs, trace=False)
    return out


# revision 5
# speedup vs baseline: 1.0086x; 1.0061x over previous
"""Trainium2 Bass kernel for nn_BiClassifier (bilinear pairwise MLP).

Math (per batch b):  out[n,m,:] = Wo @ relu(in1[n] + in2[m]) + bo
with in1 = input1 @ W1.T + b1 (HID=1024), in2 = input2 @ W2.T.

Identity used:       relu(a + c) = max(a, -c) + c
  => out = Wo @ max(a, -c)  +  v(m)  + bo,   v = input2 @ (Wo @ W2).T
The v term is rank-structured and computed exactly on the host (a [2,768]
matmul); the device only computes the irreducible pairwise part.

Sharding: HID is split across 8 cores (128 each; phase A per core is just
12 matmuls against 768-wide weight slices). Every core sees the full
inputs, produces all 4*128*128 pairs of max(a,-c) for its h-slice,
projects them against its Wo slice, and returns a [128, 1024] bf16
partial that the host sums over cores and un-diagonalizes.

Pairwise gen (the machine bottleneck: 8.4M elements/core through 128-lane
engines) is split across engines by n-row ranges:
 - n < 112: DVE tensor_tensor(max) over DIAGONALS of the (n,m) grid. With
   in2 negated and duplicated ([-c|-c], 256 cols), diagonal d is
   max(a[:, n], negdbl[:, d+n]): both operands have stride-1 innermost
   access (a: outer stride 0 over d; c: outer stride 1, overlapping
   windows), which qualifies for the DVE 2x_1P bf16 mode at FD=14336 (one
   instruction per batch; ~61ns per 128-pair row vs ~163ns for the
   baseline's per-row tensor_scalar whose per-partition scalar caps FD
   at 128).
 - n >= 112: ACT relu rows (activation Relu, in_=negdbl with scale=-1,
   bias=fp32 a-column), filling the otherwise idle ACT queue. These rows
   include the +c term, so the host applies v only to n < 112.
 (GPSIMD row gen measured pathological: ~2.8us/row tensor_scalar.)

Projection: per 512-pair chunk, matmul with a [128, 32] Wo-embedding
stationary (slot s at cols 2s:2s+2 -> psum rows 32*j4+2s:+2). Four
32-partition column groups of one [128, 512] PSUM bank are filled
concurrently via tile_position=(0, 32*j4) (4-way PE column tiling,
j4 = chunk%4 so consecutive matmuls hit different col groups), 16 slots
per group; half-bank ACT drains stream out as bf16 via both HWDGE queues,
overlapped with compute. Phase A for all batches and the ACT row-gen are
emitted up front so they execute inside the DMA-ramp window and the DVE
runs a gapless TT stream; the last batch's TT is split (96+32 diagonals)
to shorten the PE tail.

Host pre/post: transpose/shard inputs and weights (bf16), build the Wo
embeddings, un-diagonalize the output, add v and bo in fp32.
"""

import sys

import numpy as np

_REPO = "/opt/trn_rl_repo"
if _REPO not in sys.path:
    sys.path.insert(0, _REPO)

import concourse.bass as bass
import concourse.mybir as mybir
import concourse.tile as tile
from concourse import bacc
from concourse.ap import AP
from concourse.bass_utils import run_bass_kernel_spmd

B, N, D, HID, OUT = 4, 128, 768, 1024, 2
NCORES = 8
DB = D // 128           # 6 contraction blocks for phase A
G = 64                  # diagonals per tensor_tensor instruction
NTT = N // G            # TT instructions per batch
R_ACT = 16              # rows per batch generated on ACT (relu-row mode)
R_GPS = 0               # rows per batch on GPSIMD (measured pathological:
                        # gpsimd tensor_scalar with AP scalar is ~2.8us/row)
NE = N - R_ACT - R_GPS  # n-extent covered by the diagonal TTs
MM_PER_TT = G * NE // 512  # projection matmuls per TT
SLOTS = 16              # Wo-embedding slots per PSUM bank
NCHUNKS = B * N * N // 512      # 128 chunks of 512 pairs
DT = "bfloat16"

_CACHE = {}


def _build():
    f32 = mybir.dt.float32
    dt = getattr(mybir.dt, DT)
    nc = bacc.Bacc(None, target_bir_lowering=False)

    w1 = nc.declare_dram_parameter("w1", [128, DB * 128], dt, isOutput=False)
    w2 = nc.declare_dram_parameter("w2", [128, DB * 128], dt, isOutput=False)
    x1 = nc.declare_dram_parameter("x1", [128, B * DB * 128], dt, isOutput=False)
    x2 = nc.declare_dram_parameter("x2", [128, B * DB * 128], dt, isOutput=False)
    wote = nc.declare_dram_parameter("wote", [128, SLOTS * 2 * SLOTS], dt, isOutput=False)
    b1s = nc.declare_dram_parameter("b1s", [128, 1], f32, isOutput=False)
    out = nc.declare_dram_parameter("out", [128, 2 * 512], dt, isOutput=True)

    with tile.TileContext(nc) as tc:
        with (
            tc.tile_pool(name="const", bufs=1) as cpool,
            tc.tile_pool(name="hpool", bufs=3) as hpool,
            tc.tile_pool(name="apool", bufs=3) as apool,
            tc.tile_pool(name="gpool", bufs=3) as gpool,
            tc.tile_pool(name="pa1", bufs=2, space=bass.MemorySpace.PSUM) as pa1,
            tc.tile_pool(name="pa2", bufs=2, space=bass.MemorySpace.PSUM) as pa2,
            tc.tile_pool(name="po", bufs=2, space=bass.MemorySpace.PSUM) as po,
        ):
            w1sb = cpool.tile([128, DB * 128], dt)
            w2sb = cpool.tile([128, DB * 128], dt)
            # x split per batch so phase A(b) starts as soon as batch b lands.
            x1sb = cpool.tile([128, B * DB * 128], dt)
            x2sb = cpool.tile([128, B * DB * 128], dt)
            wotesb = cpool.tile([128, SLOTS * 2 * SLOTS], dt)
            b1sb = cpool.tile([128, 1], f32)
            in1t = cpool.tile([128, B * N], dt)        # a + b1, [h, (b, n)]
            RT = R_ACT + R_GPS
            in1tf = cpool.tile([128, B * RT], f32)  # fp32 a-cols for row modes
            negdbl = cpool.tile([128, B * 2 * N], dt)  # [-c | -c] per batch
            outsb = cpool.tile([128, 2 * 512], dt)

            # Ramp-critical DMAs (w1, w2, x1b0, x2b0 — everything phase A(0)
            # needs) ride the two fast HWDGE queues, 392KB each, halves
            # interleaved so the ps1 inputs land before the ps2 inputs.
            # gpsimd's slow SWDGE (~20GB/s) only carries b1s/wote/wot2,
            # which aren't needed until the first projection matmul.
            XB = DB * 128  # 768 cols per batch
            XH = XB // 2
            nc.scalar.dma_start(out=b1sb[:], in_=b1s[:])
            nc.sync.dma_start(out=w1sb[:], in_=w1[:])
            nc.scalar.dma_start(out=w2sb[:], in_=w2[:])
            nc.sync.dma_start(out=x1sb[:, 0:XH], in_=x1[:, 0:XH])
            nc.scalar.dma_start(out=x1sb[:, XH:XB], in_=x1[:, XH:XB])
            nc.sync.dma_start(out=x2sb[:, 0:XH], in_=x2[:, 0:XH])
            nc.scalar.dma_start(out=x2sb[:, XH:XB], in_=x2[:, XH:XB])
            nc.sync.dma_start(out=wotesb[:], in_=wote[:])

            # Warm the ACT table (Copy set) during the DMA fill window.
            warm = cpool.tile([128, 1], f32, name="warm")
            nc.vector.memset(warm[:], 0.0)
            nc.scalar.mul(warm[:], warm[:], -1.0)

            def phase_a(b):
                ps1 = pa1.tile([128, N], f32, tag="ps1")
                for db in range(DB):
                    nc.tensor.matmul(
                        ps1[:],
                        w1sb[:, db * 128 : (db + 1) * 128],
                        x1sb[:, (b * DB + db) * 128 : (b * DB + db + 1) * 128],
                        start=(db == 0),
                        stop=(db == DB - 1),
                    )
                ps2 = pa2.tile([128, N], f32, tag="ps2")
                for db in range(DB):
                    nc.tensor.matmul(
                        ps2[:],
                        w2sb[:, db * 128 : (db + 1) * 128],
                        x2sb[:, (b * DB + db) * 128 : (b * DB + db + 1) * 128],
                        start=(db == 0),
                        stop=(db == DB - 1),
                    )
                return ps1, ps2

            def aux(b, ps1, ps2):
                # All evacs on ACT so DVE runs a near-pure TT stream:
                # in1t slice (bf16, += b1); negated doubled in2 written as a
                # single FD=256 op (psum source read twice via 0-stride dim).
                nc.scalar.add(in1t[:, b * N : (b + 1) * N], ps1[:], b1sb[:, 0:1])
                nc.scalar.add(
                    in1tf[:, b * RT : (b + 1) * RT],
                    ps1[:, NE:N],
                    b1sb[:, 0:1],
                )
                d_sl = negdbl[:, b * 2 * N : (b + 1) * 2 * N]
                d_ap = AP(d_sl.tensor, d_sl.offset,
                          [list(d_sl.ap[0]), [N, 2], [1, N]])
                s_sl = ps2[:]
                s_ap = AP(s_sl.tensor, s_sl.offset,
                          [list(s_sl.ap[0]), [0, 2], [1, N]])
                nc.scalar.mul(d_ap, s_ap, -1.0)

            def act_gen(b):
                # R_ACT relu rows (n >= NE+R_GPS) on ACT: relu(c + a_n) read
                # from negdbl with scale=-1, bias = fp32 a-column.
                at = apool.tile([128, R_ACT * N], dt, tag="act")
                for k in range(R_ACT):
                    i = b * RT + R_GPS + k
                    nc.scalar.activation(
                        at[:, k * N : (k + 1) * N],
                        negdbl[:, b * 2 * N : b * 2 * N + N],
                        mybir.ActivationFunctionType.Relu,
                        bias=in1tf[:, i : i + 1],
                        scale=-1.0,
                    )
                return at

            def gps_gen(b):
                # R_GPS max rows (NE <= n < NE+R_GPS) on GPSIMD:
                # max(-c, a_n) — same max semantics as the diagonals.
                if not R_GPS:
                    return None
                gt = gpool.tile([128, R_GPS * N], dt, tag="gps")
                for k in range(R_GPS):
                    i = b * RT + k
                    nc.gpsimd.tensor_scalar(
                        gt[:, k * N : (k + 1) * N],
                        negdbl[:, b * 2 * N : b * 2 * N + N],
                        in1tf[:, i : i + 1],
                        None,
                        mybir.AluOpType.max,
                    )
                return gt

            # Phase A for all batches runs entirely inside the DMA-ramp
            # window (PE is otherwise idle there), so the main loop is a
            # pure TT -> projection pipeline. The non-critical x DMAs are
            # emitted after aux(0) so their issue cost doesn't sit ahead of
            # aux(0) in the in-order ACT queue.
            act_tiles = {}
            gps_tiles = {}
            cur = phase_a(0)
            aux(0, *cur)
            act_tiles[0] = act_gen(0)
            gps_tiles[0] = gps_gen(0)
            nc.sync.dma_start(out=x1sb[:, XB : 2 * XB], in_=x1[:, XB : 2 * XB])
            nc.scalar.dma_start(out=x2sb[:, XB : 2 * XB], in_=x2[:, XB : 2 * XB])
            nc.sync.dma_start(out=x2sb[:, 2 * XB :], in_=x2[:, 2 * XB :])
            nc.scalar.dma_start(out=x1sb[:, 2 * XB :], in_=x1[:, 2 * XB :])
            for b in range(1, B):
                cur = phase_a(b)
                aux(b, *cur)
                act_tiles[b] = act_gen(b)
                gps_tiles[b] = gps_gen(b)

            # Projection: one [128, 512] PSUM bank per 64 chunks, split into
            # four 32-partition column groups (tile_position from the psum
            # slice base) so four M=32 matmuls run concurrently in the PE
            # array. chunk c: bank t=c//64, colgroup j4=(c%64)%4 (j4 fastest
            # so consecutive matmuls hit different col groups), slot s=
            # (c%64)//4 picks the Wo embedding and psum rows 32*j4+2*s:+2.
            c_global = 0
            pso = None

            def project(src_ap, cc63_drain=True):
                nonlocal c_global, pso
                t, cc = c_global // 64, c_global % 64
                j4, s = cc % 4, cc // 4
                if cc == 0:
                    pso = po.tile([128, 512], f32, tag="pso")
                nc.tensor.matmul(
                    pso[32 * j4 : 32 * (j4 + 1), :],
                    wotesb[:, s * 2 * SLOTS : (s + 1) * 2 * SLOTS],
                    src_ap,
                    start=(s == 0),
                    stop=(s == SLOTS - 1),
                    tile_position=(0, 32 * j4),
                )
                return t, cc

            def drain(t, half):
                # Half-bank drain (col groups 0-1 after cc==61, 2-3 after
                # cc==63) + its DMA, overlapped with the remaining compute.
                p0, p1 = (0, 64) if half == 0 else (64, 128)
                q = nc.sync if half == 0 else nc.scalar
                nc.scalar.copy(
                    outsb[p0:p1, t * 512 : (t + 1) * 512],
                    pso[p0:p1, :],
                )
                q.dma_start(
                    out=out[p0:p1, t * 512 : (t + 1) * 512],
                    in_=outsb[p0:p1, t * 512 : (t + 1) * 512],
                )

            # Per batch: the ACT-generated chunks are consumed FIRST (their
            # gen runs early on the otherwise-idle ACT queue), so the final
            # chunks of the kernel come straight from the last TT and the
            # stream never ends waiting on ACT.
            for b in range(B):
                at = act_tiles[b]
                for j in range(R_ACT * N // 512):
                    t, cc = project(at[:, j * 512 : (j + 1) * 512])
                    if cc == 61:
                        drain(t, 0)
                    elif cc == 63:
                        drain(t, 1)
                    c_global += 1
                gt = gps_tiles[b]
                for j in range(R_GPS * N // 512):
                    t, cc = project(gt[:, j * 512 : (j + 1) * 512])
                    if cc == 61:
                        drain(t, 0)
                    elif cc == 63:
                        drain(t, 1)
                    c_global += 1
                # One big TT per batch; the very last is split so the PE
                # tail after the final TT is short.
                segs = [(0, 96), (96, 32)] if b == B - 1 else [(0, N)]
                for d0, g in segs:
                    gen = hpool.tile([128, N * NE], dt, tag="gen",
                                     name="gen")[:, : g * NE]
                    a_sl = in1t[:, b * N : b * N + NE]
                    a_ap = AP(
                        a_sl.tensor, a_sl.offset,
                        [list(a_sl.ap[0]), [0, g], [1, NE]],
                    )
                    c_sl = negdbl[:, b * 2 * N + d0 : b * 2 * N + d0 + NE]
                    c_ap = AP(
                        c_sl.tensor, c_sl.offset,
                        [list(c_sl.ap[0]), [1, g], [1, NE]],
                    )
                    g_ap = AP(
                        gen.tensor, gen.offset,
                        [list(gen.ap[0]), [NE, g], [1, NE]],
                    )
                    nc.vector.tensor_tensor(g_ap, a_ap, c_ap, mybir.AluOpType.max)
                    for j in range(g * NE // 512):
                        t, cc = project(gen[:, j * 512 : (j + 1) * 512])
                        if cc == 61:
                            drain(t, 0)
                        elif cc == 63:
                            drain(t, 1)
                        c_global += 1

    nc.finalize()
    return nc


def _np_dt():
    import ml_dtypes

    return ml_dtypes.bfloat16


def _host_prep(input1, input2, W1, b1, W2, Wo):
    f32 = np.float32
    dt = _np_dt()
    c = np.ascontiguousarray

    x1 = c(input1.reshape(B, N, DB, 128).transpose(3, 0, 2, 1).reshape(128, -1), dt)
    x2 = c(input2.reshape(B, N, DB, 128).transpose(3, 0, 2, 1).reshape(128, -1), dt)
    in_maps = []
    for core in range(NCORES):
        h0 = core * 128
        w1k = c(W1[h0:h0+128].reshape(128, DB, 128).transpose(2, 1, 0).reshape(128, -1), dt)
        w2k = c(W2[h0:h0+128].reshape(128, DB, 128).transpose(2, 1, 0).reshape(128, -1), dt)
        wotek = np.zeros((128, SLOTS, 2 * SLOTS), f32)
        for s in range(SLOTS):
            wotek[:, s, 2 * s] = Wo[0, h0:h0+128]
            wotek[:, s, 2 * s + 1] = Wo[1, h0:h0+128]
        in_maps.append({
            "w1": w1k, "w2": w2k, "x1": x1, "x2": x2,
            "wote": c(wotek.reshape(128, -1), dt),
            "b1s": c(b1[h0:h0+128].reshape(128, 1), f32),
        })
    return in_maps


_POST_IDX = None


def _post_indices():
    global _POST_IDX
    if _POST_IDX is None:
        cc = np.arange(NCHUNKS)
        col = np.arange(512)
        nact = R_ACT * N // 512
        ngps = R_GPS * N // 512
        per_b = nact + ngps + NTT * MM_PER_TT
        b_l, n_l, m_l = [], [], []
        for c in range(NCHUNKS):
            b, local = c // per_b, c % per_b
            if local < nact:
                n = NE + R_GPS + (local * 512 + col) // N
                m = col % N
            elif local < nact + ngps:
                n = NE + ((local - nact) * 512 + col) // N
                m = col % N
            else:
                local2 = local - nact - ngps
                tt, j = local2 // MM_PER_TT, local2 % MM_PER_TT
                flat = j * 512 + col
                dl, n = flat // NE, flat % NE
                m = (n + tt * G + dl) % N
            b_l.append(np.full(512, b))
            n_l.append(n)
            m_l.append(m)
        b_full, n_idx, m_idx = np.array(b_l), np.array(n_l), np.array(m_l)
        # psum location: value(c, col, o) = summed[32*j4 + 2*s + o, t*512+col]
        t_i, c64 = cc // 64, cc % 64
        j4_i, s_i = c64 % 4, c64 // 4
        row0 = 32 * j4_i + 2 * s_i
        src_col = t_i[:, None] * 512 + col[None, :]
        _POST_IDX = (b_full, n_idx, m_idx, row0, src_col)
    return _POST_IDX


def _host_post(results, varr, bo):
    summed = np.zeros((128, 2 * 512), np.float32)
    for core in range(NCORES):
        summed += np.asarray(results[core]["out"], np.float32)

    b_full, n_idx, m_idx, row0, src_col = _post_indices()
    vals = np.stack(
        [summed[row0[:, None], src_col], summed[row0[:, None] + 1, src_col]],
        axis=-1,
    )
    out_full = np.empty((B, N, N, OUT), np.float32)
    out_full[b_full, n_idx, m_idx] = vals
    # v correction (v = in2 @ (Wo @ W2).T, exact, host-side) applies to the
    # max-semantics diag rows; the ACT relu rows already include the +c term.
    out_full[:, : NE + R_GPS] += varr[:, None, :, :]
    out_full += np.asarray(bo, np.float32)
    return out_full


def run(inputs, trace=False, **spmd_kwargs):
    spmd_kwargs.pop("dt_name", None)
    spmd_kwargs.pop("add_w", None)
    if "nc" not in _CACHE:
        _CACHE["nc"] = _build()
    nc = _CACHE["nc"]
    in_maps = _host_prep(
        np.asarray(inputs["input1"], np.float32),
        np.asarray(inputs["input2"], np.float32),
        np.asarray(inputs["W1"], np.float32),
        np.asarray(inputs["b1"], np.float32),
        np.asarray(inputs["W2"], np.float32),
        np.asarray(inputs["Wo"], np.float32),
    )
    res = run_bass_kernel_spmd(
        nc, in_maps, list(range(NCORES)), trace=trace, **spmd_kwargs
    )
    # v[b, m, o] = sum_h Wo[o, h] * (input2 @ W2.T)[b, m, h]
    #            = input2 @ (Wo @ W2).T  — tiny, exact, on the host.
    vmat = np.asarray(inputs["Wo"], np.float32) @ np.asarray(inputs["W2"], np.float32)
    varr = np.einsum(
        "bmd,od->bmo", np.asarray(inputs["input2"], np.float32), vmat
    )
    out = _host_post(res.results, varr, np.asarray(inputs["bo"], np.float32))
    return out, res


def kernel(**inputs) -> np.ndarray:
    out, _ = run(inputs, trace=False)
    return out
